# revision 19
# baseline (speedup 1.0000x reference)
"""PreconditionerSparseUNet on 8 TRN2 NeuronCores.

Sharding: data-parallel over batch (8 images, 1 per core). Each core runs the
full U-Net on its own 512x512x1 image; weights are replicated.

v2 design notes (vs the first working version):
- Feature maps in DRAM as [crows, flat] fp16, flat = padded row-major spatial
  (Wp = W+2) with GUARD margins. crows = 32 for 16-channel maps so SBUF-side
  store partitions are dense.
- Work is organized in "slabs": each 128-partition PSUM/act tile packs
  pack = 128/stride channel-blocks, and block pk covers a CONTIGUOUS pixel
  (or row) subrange. Stores then have multi-KB contiguous runs per channel
  and one dma_start covers a whole slab.
- Halo columns are zero-punched in SBUF (strided memsets) before the store,
  so no per-element column zeroing DMAs exist.
- Loads are merged per slab: DRAM-side 3-dim APs enumerate (shift-block,
  channel, span); SBUF side is one dense partition range.
- All weights / biases are concatenated host-side into single tensors.
- Loads issue on sync (SP), stores on scalar (ACT) - two HWDGE queues.
"""

import os

import numpy as np

import concourse.bass as bass
import concourse.bacc as bacc
import concourse.mybir as mybir
from concourse.tile import TileContext
from concourse.bass_utils import run_bass_kernel_spmd

AF = mybir.ActivationFunctionType
F32 = mybir.dt.float32
F16 = mybir.dt.float16

STREAM_DT = F16
STREAM_NP = np.float16

N = 512
B = 8
ALPHA = 0.01
GUARD = 2560

CH = [1, 16, 32, 64, 128, 1]


def wp(w):
    return w + 2


def buf_flat(w):
    return wp(w) * wp(w) + 2 * GUARD


# ----------------------------------------------------------------------------
# Matmul plans (identical tap algebra to v1)
# ----------------------------------------------------------------------------

def s1_plan(cin, w):
    W = wp(w)
    if cin == 1:
        blocks = [ky * W + kx for ky in range(3) for kx in range(3)]
        mms = [dict(p0=0, segs=[(ky, kx) for ky in range(3) for kx in range(3)], r=0)]
    elif cin <= 32:
        blocks = [0, 1, 2]
        mms = [dict(p0=0, segs=[(ky, 0), (ky, 1), (ky, 2)], r=ky * W)
               for ky in range(3)]
    elif cin == 64:
        blocks = [0, 1]
        mms = []
        for ky in range(3):
            mms.append(dict(p0=0, segs=[(ky, 0), (ky, 1)], r=ky * W))
            mms.append(dict(p0=0, segs=[(ky, 2)], r=ky * W + 2))
    else:
        raise ValueError(cin)
    return blocks, mms


def s2_plan(cin, w_in):
    return s1_plan(cin, w_in)


def pmap(parity, d):
    if parity == 0:
        return 1 if d == 0 else None
    return 0 if d == 0 else 2


def tconv_plan(cin, w_in):
    W = wp(w_in)
    if cin == 128:
        blocks = [0]
    elif cin == 64:
        blocks = [0, 1]
    elif cin == 32:
        blocks = [0, 1, W, W + 1]
    else:
        raise ValueError(cin)
    classes = []
    for py in range(2):
        for px in range(2):
            dis = [d for d in range(2) if pmap(py, d) is not None]
            djs = [d for d in range(2) if pmap(px, d) is not None]
            mms = []
            if cin == 128:
                for di in dis:
                    for dj in djs:
                        mms.append(dict(p0=0, segs=[(pmap(py, di), pmap(px, dj))],
                                        r=di * W + dj))
            elif cin == 64:
                for di in dis:
                    if len(djs) == 2:
                        mms.append(dict(p0=0,
                                        segs=[(pmap(py, di), pmap(px, 0)),
                                              (pmap(py, di), pmap(px, 1))],
                                        r=di * W))
                    else:
                        mms.append(dict(p0=0, segs=[(pmap(py, di), 1)],
                                        r=di * W))
            else:  # cin == 32
                if py == 0 and px == 0:
                    mms = [dict(p0=0, segs=[(1, 1)], r=0)]
                elif py == 0 and px == 1:
                    mms = [dict(p0=0, segs=[(1, 0), (1, 2)], r=0)]
                elif py == 1 and px == 0:
                    mms = [dict(p0=0, segs=[(0, 1), None, (2, 1), None], r=0)]
                else:
                    mms = [dict(p0=0, segs=[(0, 0), (0, 2), (2, 0), (2, 2)], r=0)]
            classes.append((py, px, mms))
    return blocks, classes


# Layer table: (name, kind, cin, cout, w_in, w_out, wsrc, in, out, skip)
LAYERS = [
    ("enc1", "s1", 1, 16, 512, 512, "w_enc1", "xp", "enc1p", None),
    ("down1", "s2", 16, 32, 512, 256, "w_down1", "enc1p", "down1p", None),
    ("enc2", "s1", 32, 32, 256, 256, "w_enc2", "down1p", "enc2p", None),
    ("down2", "s2", 32, 64, 256, 128, "w_down2", "enc2p", "down2p", None),
    ("enc3", "s1", 64, 64, 128, 128, "w_enc3", "down2p", "enc3p", None),
    ("bn", "s2", 64, 128, 128, 64, "w_bn", "enc3p", "bnp", None),
    ("up2", "tc", 128, 64, 64, 128, "w_up2", "bnp", "up2p", "enc3p"),
    ("dec2", "s1", 64, 64, 128, 128, "w_dec2", "up2p", "dec2p", None),
    ("up1", "tc", 64, 32, 128, 256, "w_up1", "dec2p", "up1p", "enc2p"),
    ("dec1", "s1", 32, 32, 256, 256, "w_dec1", "up1p", "dec1p", None),
    ("up0", "tc", 32, 16, 256, 512, "w_up0", "dec1p", "up0p", "enc1p"),
    ("dec0", "s1", 16, 16, 512, 512, "w_dec0", "up0p", "dec0p", None),
]

BUF_CH = {"xp": 1, "enc1p": 16, "down1p": 32, "enc2p": 32, "down2p": 64,
          "enc3p": 64, "bnp": 128, "up2p": 64, "dec2p": 64, "up1p": 32,
          "dec1p": 32, "up0p": 16, "dec0p": 16}
BUF_W = {"xp": 512, "enc1p": 512, "down1p": 256, "enc2p": 256, "down2p": 128,
         "enc3p": 128, "bnp": 64, "up2p": 128, "dec2p": 128, "up1p": 256,
         "dec1p": 256, "up0p": 512, "dec0p": 512}


def crows(c):
    return c


def pack_stride(cout):
    return 32 if cout <= 32 else (64 if cout == 64 else 128)


def layer_plan(kind, cin, w_in):
    if kind == "s1":
        return s1_plan(cin, w_in)
    if kind == "s2":
        return s2_plan(cin, w_in)
    return tconv_plan(cin, w_in)


def mm_keys(name, kind, cin, w_in):
    out = []
    if kind in ("s1", "s2"):
        _, mms = layer_plan(kind, cin, w_in)
        for i, m in enumerate(mms):
            out.append((f"W_{name}_{i}", m))
    else:
        _, classes = layer_plan(kind, cin, w_in)
        for py, px, mms in classes:
            for i, m in enumerate(mms):
                out.append((f"W_{name}_c{py}{px}_{i}", m))
    return out


def weight_layout():
    """Column offsets of every lhsT slice inside the concatenated weight
    tensor, and bias column index per layer."""
    woff = {}
    col = 0
    for (name, kind, cin, cout, w_in, *_r) in LAYERS:
        for key, m in mm_keys(name, kind, cin, w_in):
            woff[key] = (col, len(m["segs"]) * cin, cout)
            col += cout
    woff["W_out"] = (col, 16, 1)
    col += 1
    boff = {}
    for i, (name, *_r) in enumerate(LAYERS):
        boff[name] = i
    boff["out"] = len(LAYERS)
    return woff, col, boff, len(LAYERS) + 1


WOFF, WCOLS, BOFF, BCOLS = weight_layout()


# ----------------------------------------------------------------------------
# Host-side input prep
# ----------------------------------------------------------------------------

def prep_weights(inputs):
    wcat = np.zeros((128, WCOLS), STREAM_NP)
    for (name, kind, cin, cout, w_in, w_out, wsrc, *_rest) in LAYERS:
        w = np.asarray(inputs[wsrc])  # [3,3,cin,cout]
        for key, m in mm_keys(name, kind, cin, w_in):
            segs = []
            for s in m["segs"]:
                if s is None:
                    segs.append(np.zeros((cin, cout), np.float32))
                else:
                    segs.append(w[s[0], s[1]])
            arr = np.concatenate(segs, axis=0).astype(STREAM_NP)
            col, k, co = WOFF[key]
            wcat[0:k, col:col + co] = arr
    col, k, co = WOFF["W_out"]
    wcat[0:16, col:col + 1] = np.asarray(inputs["w_out"]).reshape(16, 1)

    bcat = np.zeros((128, BCOLS), np.float32)
    for (name, kind, cin, cout, *_r) in LAYERS:
        bsrc = "b_" + name
        b = np.asarray(inputs[bsrc]).astype(np.float32)
        stride = pack_stride(cout)
        for pk in range(128 // stride):
            bcat[pk * stride: pk * stride + cout, BOFF[name]] = b
    bcat[:, BOFF["out"]] = float(np.asarray(inputs["b_out"])[0])
    return {"wcat": np.ascontiguousarray(wcat),
            "bcat": np.ascontiguousarray(bcat)}


def prep_x(img):
    """img [512,512] f32 -> [9, flat] fp16: row b holds the padded image
    shifted left by the enc1 tap-block offset s_b, so the enc1 rhs blocks
    load as one dense-partition DMA."""
    W = wp(512)
    flat = buf_flat(512)
    buf = np.zeros(flat, STREAM_NP)
    p = np.pad(img.astype(STREAM_NP), 1)
    buf[GUARD:GUARD + W * W] = p.reshape(-1)
    shifts = [ky * W + kx for ky in range(3) for kx in range(3)]
    x9 = np.zeros((9, flat), STREAM_NP)
    for b, s in enumerate(shifts):
        x9[b, :flat - s] = buf[s:]
    return x9


# ----------------------------------------------------------------------------
# Kernel builder
# ----------------------------------------------------------------------------

def sub_ap(base_ap, p0, np_, off, dims):
    pitch = base_ap.ap[0][0]
    return bass.AP(base_ap.tensor, base_ap.offset + p0 * pitch + off,
                   [[pitch, np_]] + [list(d) for d in dims])


def dram_ap(t_ap, off, dims):
    return bass.AP(t_ap.tensor, t_ap.offset + off, [list(d) for d in dims])


def build_unet():
    nc = bacc.Bacc("TRN2", target_bir_lowering=False, debug=False)

    xp_in = nc.dram_tensor("xp", [9, buf_flat(512)], STREAM_DT,
                           kind="ExternalInput").ap()
    out_t = nc.dram_tensor("out", [N, N], F32, kind="ExternalOutput").ap()
    wcat_in = nc.dram_tensor("wcat", [128, WCOLS], STREAM_DT,
                             kind="ExternalInput").ap()
    bcat_in = nc.dram_tensor("bcat", [128, BCOLS], F32,
                             kind="ExternalInput").ap()

    bufs = {}
    for nm in BUF_CH:
        if nm == "xp":
            continue
        bufs[nm] = nc.dram_tensor(nm, [crows(BUF_CH[nm]), buf_flat(BUF_W[nm])],
                                  STREAM_DT, kind="Internal").ap()

    mask_np = np.tril(np.ones((N, N), np.float32))
    mask_t = nc.inline_tensor(mask_np, name="trimask").ap()

    nlayers = int(os.environ.get("UNET_NLAYERS", "99"))

    with TileContext(nc) as tc:
        with (
            tc.tile_pool(name="wpool", bufs=1) as wpool,
            tc.tile_pool(name="tinpool", bufs=2) as tinpool,
            tc.tile_pool(name="actpool", bufs=3) as actpool,
            tc.tile_pool(name="skpool", bufs=2) as skpool,
            tc.tile_pool(name="psum", bufs=4, space="PSUM") as pspool,
        ):
            wt = wpool.tile([128, WCOLS], STREAM_DT, tag="wt")
            nc.sync.dma_start(out=wt[:, :], in_=wcat_in)
            bt = wpool.tile([128, BCOLS], F32, tag="bt")
            nc.sync.dma_start(out=bt[:, :], in_=bcat_in)
            ZW = 2700
            zt = wpool.tile([128, ZW], STREAM_DT, tag="zeros")
            nc.vector.memset(zt[:, :], 0.0)

            def wslice(key):
                col, k, co = WOFF[key]
                return wt[0:k, col:col + co]

            def bslice(name):
                return bt[:, BOFF[name]:BOFF[name] + 1]

            # --- zero top/bottom halo rows of every internal buffer -------
            for nm, bap in bufs.items():
                c, w = BUF_CH[nm], BUF_W[nm]
                W = wp(w)
                cr = crows(c)
                nc.scalar.dma_start(out=sub_ap(bap, 0, cr, GUARD, [[1, W]]),
                                    in_=zt[0:cr, 0:W])
                nc.scalar.dma_start(
                    out=sub_ap(bap, 0, cr, GUARD + (W - 1) * W, [[1, W]]),
                    in_=zt[0:cr, 0:W])

            # ---------------------------------------------------------------
            def punch(act_ap, p0, pstride, base_px, Q, W):
                """Zero halo-col positions inside a flat act range."""
                pitch = act_ap.ap[0][0]
                for edge in (0, W - 1):
                    o0 = (edge - base_px) % W
                    cnt = (Q - o0 + W - 1) // W if o0 < Q else 0
                    if cnt <= 0:
                        continue
                    ap = bass.AP(act_ap.tensor,
                                 act_ap.offset + p0 * pitch + o0,
                                 [[pitch, pstride], [W, cnt]])
                    nc.vector.memset(ap, 0.0)

            # --- s1: 3x3 stride-1 conv ------------------------------------
            def emit_s1(name, cin, cout, w, inb, outb):
                W = wp(w)
                blocks, mms = s1_plan(cin, w)
                nb = len(blocks)
                stride = pack_stride(cout)
                pack = 128 // stride
                total = w * W
                o_base = GUARD + W
                nchunks = (total + 511) // 512
                inap = xp_in if inb == "xp" else bufs[inb]
                in_pitch = inap.ap[0][0]
                bias = bslice(name)
                g_ch = 0
                while g_ch < nchunks:
                    nch = min(32, nchunks - g_ch)
                    nbk = min(pack, nch)
                    nchb = (nch + nbk - 1) // nbk
                    Q = nchb * 512
                    g = g_ch * 512
                    span = nbk * Q + 2 * W + 2
                    A0 = o_base + g - W - 1
                    ext = 0 if cin == 1 else blocks[-1]
                    tin = tinpool.tile([nb * cin, span + ext], STREAM_DT,
                                       tag="tin")
                    tin_ap = tin[:, :]
                    if cin == 1:
                        # xp9 rows are pre-shifted: one dense-partition load
                        nc.sync.dma_start(
                            out=tin[:, 0:span],
                            in_=dram_ap(inap, A0, [[in_pitch, 9], [1, span]]))
                    else:
                        # load block 0 (extended), replicate shifts on-chip
                        nc.sync.dma_start(
                            out=tin[0:cin, :],
                            in_=dram_ap(inap, A0,
                                        [[in_pitch, cin], [1, span + ext]]))
                        for b, s in enumerate(blocks[1:], start=1):
                            nc.sync.dma_start(
                                out=tin[b * cin:(b + 1) * cin, 0:span],
                                in_=tin[0:cin, s:s + span])

                    # act layout: block pk at partitions [pk*stride], free [0,Q)
                    act = actpool.tile([128, Q], STREAM_DT, tag="act")
                    act_ap = act[:, :]
                    npt = (Q + 1023) // 1024
                    for jt in range(npt):
                        ps = pspool.tile([128, 1024], F32, tag="ps")
                        nu = min(2, (Q - jt * 1024) // 512)
                        for u in range(nu):
                            for mi, m in enumerate(mms):
                                K = len(m["segs"]) * cin
                                for pk in range(nbk):
                                    foff = pk * Q + jt * 1024 + u * 512 + m["r"]
                                    rhs = sub_ap(tin_ap, 0, K, foff, [[1, 512]])
                                    nc.tensor.matmul(
                                        ps[pk * stride:pk * stride + cout,
                                           u * 512:u * 512 + 512],
                                        lhsT=wslice(f"W_{name}_{mi}"),
                                        rhs=rhs,
                                        start=(mi == 0), stop=(mi == len(mms) - 1),
                                        tile_position=(0, pk * stride))
                        tl = nu * 512
                        nc.scalar.activation(
                            sub_ap(act_ap, 0, 128, jt * 1024, [[1, tl]]),
                            ps[:, 0:tl], AF.Prelu, bias=bias, alpha=ALPHA)
                    for pk in range(nbk):
                        punch(act_ap, pk * stride, stride, g + pk * Q, Q, W)
                    # per-block stores: outer dim = cout spreads engines
                    dp = bufs[outb].ap[0][0]
                    for pk in range(nbk):
                        nc.sync.dma_start(
                            out=dram_ap(bufs[outb], o_base + g + pk * Q,
                                        [[dp, cout], [1, Q]]),
                            in_=act[pk * stride:pk * stride + cout, 0:Q])
                    g_ch += nbk * nchb
                grid_end = g_ch * 512
                if grid_end > total:
                    ov = grid_end - total
                    ln = max(W, ov)
                    nc.scalar.dma_start(
                        out=sub_ap(bufs[outb], 0, cout, o_base + total, [[1, ln]]),
                        in_=zt[0:cout, 0:ln])

            # --- s2: 3x3 stride-2 conv ------------------------------------
            def emit_s2(name, cin, cout, w_in, w_out, inb, outb):
                Wi, Wo = wp(w_in), wp(w_out)
                blocks, mms = s2_plan(cin, w_in)
                nb = len(blocks)
                stride = pack_stride(cout)
                pack = 128 // stride
                Rq = 1024 // w_out
                Rch = 512 // w_out
                SR = pack * Rq
                L = Rq * Wo
                in_pitch = bufs[inb].ap[0][0]
                bias = bslice(name)
                for y0 in range(0, w_out, SR):
                    A0 = GUARD + 2 * y0 * Wi
                    span = (2 * SR + 2) * Wi
                    ext = blocks[-1]
                    tin = tinpool.tile([nb * cin, span + ext], STREAM_DT,
                                       tag="tin")
                    tin_ap = tin[:, :]
                    nc.sync.dma_start(
                        out=tin[0:cin, :],
                        in_=dram_ap(bufs[inb], A0,
                                    [[in_pitch, cin], [1, span + ext]]))
                    for b, s in enumerate(blocks[1:], start=1):
                        nc.sync.dma_start(
                            out=tin[b * cin:(b + 1) * cin, 0:span],
                            in_=tin[0:cin, s:s + span])

                    act = actpool.tile([128, L], STREAM_DT, tag="act")
                    act_ap = act[:, :]
                    ps = pspool.tile([128, 1024], F32, tag="ps")
                    for u in range(2):
                        for mi, m in enumerate(mms):
                            K = len(m["segs"]) * cin
                            for pk in range(pack):
                                foff = 2 * (pk * Rq + u * Rch) * Wi + m["r"]
                                rhs = sub_ap(tin_ap, 0, K, foff,
                                             [[2 * Wi, Rch], [2, w_out]])
                                nc.tensor.matmul(
                                    ps[pk * stride:pk * stride + cout,
                                       u * 512:u * 512 + 512],
                                    lhsT=wslice(f"W_{name}_{mi}"),
                                    rhs=rhs,
                                    start=(mi == 0), stop=(mi == len(mms) - 1),
                                    tile_position=(0, pk * stride))
                    nc.scalar.activation(
                        bass.AP(act_ap.tensor, act_ap.offset + 1,
                                [[act_ap.ap[0][0], 128], [Wo, Rq], [1, w_out]]),
                        ps[:, 0:1024], AF.Prelu, bias=bias, alpha=ALPHA)
                    nc.vector.memset(
                        bass.AP(act_ap.tensor, act_ap.offset,
                                [[act_ap.ap[0][0], 128], [Wo, Rq]]), 0.0)
                    nc.vector.memset(
                        bass.AP(act_ap.tensor, act_ap.offset + Wo - 1,
                                [[act_ap.ap[0][0], 128], [Wo, Rq]]), 0.0)
                    dp = bufs[outb].ap[0][0]
                    for pk in range(pack):
                        nc.sync.dma_start(
                            out=dram_ap(bufs[outb],
                                        GUARD + (1 + y0 + pk * Rq) * Wo,
                                        [[dp, cout], [1, L]]),
                            in_=act[pk * stride:pk * stride + cout, 0:L])

            # --- tconv: stride-2 transposed conv + skip-add ---------------
            def emit_tconv(name, cin, cout, w_in, w_out, inb, outb, skipb):
                Wi, Wop = wp(w_in), wp(w_out)
                blocks, classes = tconv_plan(cin, w_in)
                nb = len(blocks)
                stride = pack_stride(cout)
                pack = 128 // stride
                Ri = 512 // w_in
                Rq = 1024 // w_in
                SR = pack * Rq
                L = 2 * Rq * Wop
                in_pitch = bufs[inb].ap[0][0]
                sk_pitch = bufs[skipb].ap[0][0]
                bias = bslice(name)
                for i0 in range(0, w_in, SR):
                    A0 = GUARD + (1 + i0) * Wi + 1
                    span = (SR + 2) * Wi
                    ext = blocks[-1]
                    tin = tinpool.tile([nb * cin, span + ext], STREAM_DT,
                                       tag="tin")
                    tin_ap = tin[:, :]
                    nc.sync.dma_start(
                        out=tin[0:cin, :],
                        in_=dram_ap(bufs[inb], A0,
                                    [[in_pitch, cin], [1, span + ext]]))
                    for b, s in enumerate(blocks[1:], start=1):
                        nc.sync.dma_start(
                            out=tin[b * cin:(b + 1) * cin, 0:span],
                            in_=tin[0:cin, s:s + span])

                    slab = actpool.tile([128, L], STREAM_DT, tag="act")
                    slab_ap = slab[:, :]
                    spitch = slab_ap.ap[0][0]
                    for py in range(2):
                        for px in range(2):
                            mms = [m for (py_, px_, cmms) in classes
                                   if (py_, px_) == (py, px) for m in cmms]
                            ps = pspool.tile([128, 1024], F32, tag="ps")
                            for u in range(2):
                                for mi, m in enumerate(mms):
                                    K = len(m["segs"]) * cin
                                    for pk in range(pack):
                                        foff = (pk * Rq + u * Ri) * Wi + m["r"]
                                        rhs = sub_ap(tin_ap, 0, K, foff,
                                                     [[Wi, Ri], [1, w_in]])
                                        nc.tensor.matmul(
                                            ps[pk * stride:pk * stride + cout,
                                               u * 512:u * 512 + 512],
                                            lhsT=wslice(f"W_{name}_c{py}{px}_{mi}"),
                                            rhs=rhs,
                                            start=(mi == 0),
                                            stop=(mi == len(mms) - 1),
                                            tile_position=(0, pk * stride))
                            nc.scalar.activation(
                                bass.AP(slab_ap.tensor,
                                        slab_ap.offset + py * Wop + 1 + px,
                                        [[spitch, 128], [2 * Wop, Rq], [2, w_in]]),
                                ps[:, 0:1024], AF.Prelu, bias=bias, alpha=ALPHA)
                    nc.vector.memset(
                        bass.AP(slab_ap.tensor, slab_ap.offset,
                                [[spitch, 128], [Wop, 2 * Rq]]), 0.0)
                    nc.vector.memset(
                        bass.AP(slab_ap.tensor, slab_ap.offset + Wop - 1,
                                [[spitch, 128], [Wop, 2 * Rq]]), 0.0)
                    skt = skpool.tile([128, L], STREAM_DT, tag="skt")
                    for pk in range(pack):
                        off = GUARD + (1 + 2 * (i0 + pk * Rq)) * Wop
                        nc.scalar.dma_start(
                            out=skt[pk * stride:pk * stride + cout, 0:L],
                            in_=dram_ap(bufs[skipb], off,
                                        [[sk_pitch, cout], [1, L]]))
                    nc.vector.tensor_add(out=slab_ap, in0=slab_ap, in1=skt[:, :])
                    dp = bufs[outb].ap[0][0]
                    for pk in range(pack):
                        off = GUARD + (1 + 2 * (i0 + pk * Rq)) * Wop
                        nc.sync.dma_start(
                            out=dram_ap(bufs[outb], off, [[dp, cout], [1, L]]),
                            in_=slab[pk * stride:pk * stride + cout, 0:L])

            for (name, kind, cin, cout, w_in, w_out, wsrc, inb, outb, skipb) \
                    in LAYERS[:nlayers]:
                if kind == "s1":
                    emit_s1(name, cin, cout, w_in, inb, outb)
                elif kind == "s2":
                    emit_s2(name, cin, cout, w_in, w_out, inb, outb)
                else:
                    emit_tconv(name, cin, cout, w_in, w_out, inb, outb, skipb)

            # --- final 1x1 conv + triangular masking ----------------------
            do_tail = nlayers > len(LAYERS)
            if not do_tail:
                nc.sync.dma_start(out=out_t[:, :], in_=mask_t[:, :])
            W0 = wp(512)
            if do_tail:
                bias = bslice("out")
                dp = bufs["dec0p"].ap[0][0]
                for y0 in range(0, 512, 32):
                    A0 = GUARD + (1 + y0) * W0 + 1
                    span = 31 * W0 + 512
                    tin = tinpool.tile([16, span], STREAM_DT, tag="tin")
                    tin_ap = tin[:, :]
                    nc.sync.dma_start(out=tin_ap,
                                      in_=dram_ap(bufs["dec0p"], A0,
                                                  [[dp, 16], [1, span]]))
                    for t in range(4):
                        ps = pspool.tile([128, 1024], F32, tag="ps")
                        for u in range(2):
                            for pk in range(4):
                                r = t * 8 + pk * 2 + u
                                rhs = sub_ap(tin_ap, 0, 16, r * W0, [[1, 512]])
                                nc.tensor.matmul(
                                    ps[pk * 32:pk * 32 + 1,
                                       u * 512:u * 512 + 512],
                                    lhsT=wslice("W_out"), rhs=rhs,
                                    start=True, stop=True,
                                    tile_position=(0, pk * 32))
                        act = actpool.tile([128, 1024], F32, tag="actf")
                        nc.scalar.activation(act[:, :], ps[:, :], AF.Identity,
                                             bias=bias)
                        mt = skpool.tile([128, 1024], F32, tag="mask")
                        mt_ap = mt[:, :]
                        mpitch = mt_ap.ap[0][0]
                        nc.scalar.dma_start(
                            out=bass.AP(mt_ap.tensor, mt_ap.offset,
                                        [[32 * mpitch, 4], [512, 2], [1, 512]]),
                            in_=dram_ap(mask_t, (y0 + t * 8) * 512,
                                        [[1024, 4], [512, 2], [1, 512]]))
                        nc.vector.tensor_mul(out=act[:, :], in0=act[:, :],
                                             in1=mt_ap)
                        a_ap = act[:, :]
                        apitch = a_ap.ap[0][0]
                        nc.sync.dma_start(
                            out=dram_ap(out_t, (y0 + t * 8) * 512,
                                        [[1024, 4], [512, 2], [1, 512]]),
                            in_=bass.AP(a_ap.tensor, a_ap.offset,
                                        [[32 * apitch, 4], [512, 2], [1, 512]]))

            # --- diagonal softplus patch ----------------------------------
            do_diag = nlayers > len(LAYERS) + 1
            if do_diag:
                out_flat = out_t.flatten()
                diag_ap = bass.AP(out_flat.tensor, out_flat.offset, [[513, 512]])
                dt_ = actpool.tile([1, 512], F32, tag="diag")
                nc.sync.dma_start(out=dt_[:, :], in_=diag_ap)
                ta = actpool.tile([1, 512], F32, tag="diag_a")
                nc.scalar.activation(ta[:, :], dt_[:, :], AF.Abs)
                nc.scalar.activation(ta[:, :], ta[:, :], AF.Exp, scale=-1.0)
                nc.vector.tensor_scalar_add(out=ta[:, :], in0=ta[:, :], scalar1=1.0)
                nc.scalar.activation(ta[:, :], ta[:, :], AF.Ln)
                tr = actpool.tile([1, 512], F32, tag="diag_r")
                nc.scalar.activation(tr[:, :], dt_[:, :], AF.Relu)
                nc.vector.tensor_add(out=tr[:, :], in0=tr[:, :], in1=ta[:, :])
                nc.sync.dma_start(out=diag_ap, in_=tr[:, :])

    nc.compile()
    return nc


_NC_CACHE = None


def get_nc():
    global _NC_CACHE
    if _NC_CACHE is None:
        _NC_CACHE = build_unet()
    return _NC_CACHE


def make_in_maps(inputs):
    wmap = prep_weights(inputs)
    x = np.asarray(inputs["x"])  # [8, 512, 512, 1] f32
    in_maps = []
    for i in range(B):
        m = dict(wmap)
        m["xp"] = prep_x(x[i, :, :, 0])
        in_maps.append(m)
    return in_maps


def kernel(_trace=False, **inputs):
    nc = get_nc()
    in_maps = make_in_maps(inputs)
    res = run_bass_kernel_spmd(nc, in_maps, core_ids=list(range(B)),
                               trace=_trace)
    out = np.stack([res.results[i]["out"] for i in range(B)], axis=0)
    out = out[:, :, :, None].astype(np.float32)
    if _trace:
        return out, res
    return out


# revision 20
# speedup vs baseline: 1.2216x; 1.2216x over previous
"""PreconditionerSparseUNet on 8 TRN2 NeuronCores.

Sharding: data-parallel over batch (8 images, 1 per core). Each core runs the
full U-Net on its own 512x512x1 image; weights are replicated.

v2 design notes (vs the first working version):
- Feature maps in DRAM as [crows, flat] fp16, flat = padded row-major spatial
  (Wp = W+2) with GUARD margins. crows = 32 for 16-channel maps so SBUF-side
  store partitions are dense.
- Work is organized in "slabs": each 128-partition PSUM/act tile packs
  pack = 128/stride channel-blocks, and block pk covers a CONTIGUOUS pixel
  (or row) subrange. Stores then have multi-KB contiguous runs per channel
  and one dma_start covers a whole slab.
- Halo columns are zero-punched in SBUF (strided memsets) before the store,
  so no per-element column zeroing DMAs exist.
- Loads are merged per slab: DRAM-side 3-dim APs enumerate (shift-block,
  channel, span); SBUF side is one dense partition range.
- All weights / biases are concatenated host-side into single tensors.
- Loads issue on sync (SP), stores on scalar (ACT) - two HWDGE queues.
"""

import os

import numpy as np

import concourse.bass as bass
import concourse.bacc as bacc
import concourse.mybir as mybir
from concourse.tile import TileContext
from concourse.bass_utils import run_bass_kernel_spmd

AF = mybir.ActivationFunctionType
F32 = mybir.dt.float32
F16 = mybir.dt.float16

STREAM_DT = F16
STREAM_NP = np.float16

N = 512
B = 8
ALPHA = 0.01
GUARD = 2560

CH = [1, 16, 32, 64, 128, 1]


def wp(w):
    return w + 2


def buf_flat(w):
    return wp(w) * wp(w) + 2 * GUARD


# ----------------------------------------------------------------------------
# Matmul plans (identical tap algebra to v1)
# ----------------------------------------------------------------------------

def s1_plan(cin, w):
    W = wp(w)
    if cin == 1:
        blocks = [ky * W + kx for ky in range(3) for kx in range(3)]
        mms = [dict(p0=0, segs=[(ky, kx) for ky in range(3) for kx in range(3)], r=0)]
    elif cin <= 32:
        blocks = [0, 1, 2]
        mms = [dict(p0=0, segs=[(ky, 0), (ky, 1), (ky, 2)], r=ky * W)
               for ky in range(3)]
    elif cin == 64:
        blocks = [0, 1]
        mms = []
        for ky in range(3):
            mms.append(dict(p0=0, segs=[(ky, 0), (ky, 1)], r=ky * W))
            mms.append(dict(p0=0, segs=[(ky, 2)], r=ky * W + 2))
    else:
        raise ValueError(cin)
    return blocks, mms


def s2_plan(cin, w_in):
    return s1_plan(cin, w_in)


def pmap(parity, d):
    if parity == 0:
        return 1 if d == 0 else None
    return 0 if d == 0 else 2


def tconv_plan(cin, w_in):
    W = wp(w_in)
    if cin == 128:
        blocks = [0]
    elif cin == 64:
        blocks = [0, 1]
    elif cin == 32:
        blocks = [0, 1, W, W + 1]
    else:
        raise ValueError(cin)
    classes = []
    for py in range(2):
        for px in range(2):
            dis = [d for d in range(2) if pmap(py, d) is not None]
            djs = [d for d in range(2) if pmap(px, d) is not None]
            mms = []
            if cin == 128:
                for di in dis:
                    for dj in djs:
                        mms.append(dict(p0=0, segs=[(pmap(py, di), pmap(px, dj))],
                                        r=di * W + dj))
            elif cin == 64:
                for di in dis:
                    if len(djs) == 2:
                        mms.append(dict(p0=0,
                                        segs=[(pmap(py, di), pmap(px, 0)),
                                              (pmap(py, di), pmap(px, 1))],
                                        r=di * W))
                    else:
                        mms.append(dict(p0=0, segs=[(pmap(py, di), 1)],
                                        r=di * W))
            else:  # cin == 32
                if py == 0 and px == 0:
                    mms = [dict(p0=0, segs=[(1, 1)], r=0)]
                elif py == 0 and px == 1:
                    mms = [dict(p0=0, segs=[(1, 0), (1, 2)], r=0)]
                elif py == 1 and px == 0:
                    mms = [dict(p0=0, segs=[(0, 1), None, (2, 1), None], r=0)]
                else:
                    mms = [dict(p0=0, segs=[(0, 0), (0, 2), (2, 0), (2, 2)], r=0)]
            classes.append((py, px, mms))
    return blocks, classes


# Layer table: (name, kind, cin, cout, w_in, w_out, wsrc, in, out, skip)
LAYERS = [
    ("enc1", "s1", 1, 16, 512, 512, "w_enc1", "xp", "enc1p", None),
    ("down1", "s2", 16, 32, 512, 256, "w_down1", "enc1p", "down1p", None),
    ("enc2", "s1", 32, 32, 256, 256, "w_enc2", "down1p", "enc2p", None),
    ("down2", "s2", 32, 64, 256, 128, "w_down2", "enc2p", "down2p", None),
    ("enc3", "s1", 64, 64, 128, 128, "w_enc3", "down2p", "enc3p", None),
    ("bn", "s2", 64, 128, 128, 64, "w_bn", "enc3p", "bnp", None),
    ("up2", "tc", 128, 64, 64, 128, "w_up2", "bnp", "up2p", "enc3p"),
    ("dec2", "s1", 64, 64, 128, 128, "w_dec2", "up2p", "dec2p", None),
    ("up1", "tc", 64, 32, 128, 256, "w_up1", "dec2p", "up1p", "enc2p"),
    ("dec1", "s1", 32, 32, 256, 256, "w_dec1", "up1p", "dec1p", None),
    ("up0", "tc", 32, 16, 256, 512, "w_up0", "dec1p", "up0p", "enc1p"),
    ("dec0", "s1", 16, 16, 512, 512, "w_dec0", "up0p", "dec0p", None),
]

BUF_CH = {"xp": 1, "enc1p": 16, "down1p": 32, "enc2p": 32, "down2p": 64,
          "enc3p": 64, "bnp": 128, "up2p": 64, "dec2p": 64, "up1p": 32,
          "dec1p": 32, "up0p": 16, "dec0p": 16}
BUF_W = {"xp": 512, "enc1p": 512, "down1p": 256, "enc2p": 256, "down2p": 128,
         "enc3p": 128, "bnp": 64, "up2p": 128, "dec2p": 128, "up1p": 256,
         "dec1p": 256, "up0p": 512, "dec0p": 512}


def crows(c):
    return c


def pack_stride(cout):
    return 32 if cout <= 32 else (64 if cout == 64 else 128)


def layer_plan(kind, cin, w_in):
    if kind == "s1":
        return s1_plan(cin, w_in)
    if kind == "s2":
        return s2_plan(cin, w_in)
    return tconv_plan(cin, w_in)


def mm_keys(name, kind, cin, w_in):
    out = []
    if kind in ("s1", "s2"):
        _, mms = layer_plan(kind, cin, w_in)
        for i, m in enumerate(mms):
            out.append((f"W_{name}_{i}", m))
    else:
        _, classes = layer_plan(kind, cin, w_in)
        for py, px, mms in classes:
            for i, m in enumerate(mms):
                out.append((f"W_{name}_c{py}{px}_{i}", m))
    return out


def weight_layout():
    """Column offsets of every lhsT slice inside the concatenated weight
    tensor, and bias column index per layer."""
    woff = {}
    col = 0
    for (name, kind, cin, cout, w_in, *_r) in LAYERS:
        for key, m in mm_keys(name, kind, cin, w_in):
            woff[key] = (col, len(m["segs"]) * cin, cout)
            col += cout
    woff["W_out"] = (col, 16, 1)
    col += 1
    boff = {}
    for i, (name, *_r) in enumerate(LAYERS):
        boff[name] = i
    boff["out"] = len(LAYERS)
    return woff, col, boff, len(LAYERS) + 1


WOFF, WCOLS, BOFF, BCOLS = weight_layout()


# ----------------------------------------------------------------------------
# Host-side input prep
# ----------------------------------------------------------------------------

def prep_weights(inputs):
    wcat = np.zeros((128, WCOLS), STREAM_NP)
    for (name, kind, cin, cout, w_in, w_out, wsrc, *_rest) in LAYERS:
        w = np.asarray(inputs[wsrc])  # [3,3,cin,cout]
        for key, m in mm_keys(name, kind, cin, w_in):
            segs = []
            for s in m["segs"]:
                if s is None:
                    segs.append(np.zeros((cin, cout), np.float32))
                else:
                    segs.append(w[s[0], s[1]])
            arr = np.concatenate(segs, axis=0).astype(STREAM_NP)
            col, k, co = WOFF[key]
            wcat[0:k, col:col + co] = arr
    col, k, co = WOFF["W_out"]
    wcat[0:16, col:col + 1] = np.asarray(inputs["w_out"]).reshape(16, 1)

    bcat = np.zeros((128, BCOLS), np.float32)
    for (name, kind, cin, cout, *_r) in LAYERS:
        bsrc = "b_" + name
        b = np.asarray(inputs[bsrc]).astype(np.float32)
        stride = pack_stride(cout)
        for pk in range(128 // stride):
            bcat[pk * stride: pk * stride + cout, BOFF[name]] = b
    bcat[:, BOFF["out"]] = float(np.asarray(inputs["b_out"])[0])
    return {"wcat": np.ascontiguousarray(wcat),
            "bcat": np.ascontiguousarray(bcat)}


def prep_x(img):
    """img [512,512] f32 -> [9, flat] fp16: row b holds the padded image
    shifted left by the enc1 tap-block offset s_b, so the enc1 rhs blocks
    load as one dense-partition DMA."""
    W = wp(512)
    flat = buf_flat(512)
    buf = np.zeros(flat, STREAM_NP)
    p = np.pad(img.astype(STREAM_NP), 1)
    buf[GUARD:GUARD + W * W] = p.reshape(-1)
    shifts = [ky * W + kx for ky in range(3) for kx in range(3)]
    x9 = np.zeros((9, flat), STREAM_NP)
    for b, s in enumerate(shifts):
        x9[b, :flat - s] = buf[s:]
    return x9


# ----------------------------------------------------------------------------
# Kernel builder
# ----------------------------------------------------------------------------

def sub_ap(base_ap, p0, np_, off, dims):
    pitch = base_ap.ap[0][0]
    return bass.AP(base_ap.tensor, base_ap.offset + p0 * pitch + off,
                   [[pitch, np_]] + [list(d) for d in dims])


def dram_ap(t_ap, off, dims):
    return bass.AP(t_ap.tensor, t_ap.offset + off, [list(d) for d in dims])


def build_unet():
    nc = bacc.Bacc("TRN2", target_bir_lowering=False, debug=False)

    xp_in = nc.dram_tensor("xp", [9, buf_flat(512)], STREAM_DT,
                           kind="ExternalInput").ap()
    out_t = nc.dram_tensor("out", [N, N], F32, kind="ExternalOutput").ap()
    wcat_in = nc.dram_tensor("wcat", [128, WCOLS], STREAM_DT,
                             kind="ExternalInput").ap()
    bcat_in = nc.dram_tensor("bcat", [128, BCOLS], F32,
                             kind="ExternalInput").ap()

    bufs = {}
    for nm in BUF_CH:
        if nm == "xp":
            continue
        bufs[nm] = nc.dram_tensor(nm, [crows(BUF_CH[nm]), buf_flat(BUF_W[nm])],
                                  STREAM_DT, kind="Internal").ap()

    mask_np = np.tril(np.ones((N, N), np.float32))
    mask_t = nc.inline_tensor(mask_np, name="trimask").ap()

    nlayers = int(os.environ.get("UNET_NLAYERS", "99"))

    with TileContext(nc) as tc:
        with (
            tc.tile_pool(name="wpool", bufs=1) as wpool,
            tc.tile_pool(name="tinpool", bufs=2) as tinpool,
            tc.tile_pool(name="actpool", bufs=3) as actpool,
            tc.tile_pool(name="skpool", bufs=2) as skpool,
            tc.tile_pool(name="psum", bufs=4, space="PSUM") as pspool,
        ):
            wt = wpool.tile([128, WCOLS], STREAM_DT, tag="wt")
            nc.sync.dma_start(out=wt[:, :], in_=wcat_in)
            bt = wpool.tile([128, BCOLS], F32, tag="bt")
            nc.sync.dma_start(out=bt[:, :], in_=bcat_in)
            ZW = 2700
            zt = wpool.tile([128, ZW], STREAM_DT, tag="zeros")
            nc.vector.memset(zt[:, :], 0.0)

            def wslice(key):
                col, k, co = WOFF[key]
                return wt[0:k, col:col + co]

            def bslice(name):
                return bt[:, BOFF[name]:BOFF[name] + 1]

            # --- zero top/bottom halo rows of every internal buffer -------
            for nm, bap in bufs.items():
                c, w = BUF_CH[nm], BUF_W[nm]
                W = wp(w)
                cr = crows(c)
                nc.scalar.dma_start(out=sub_ap(bap, 0, cr, GUARD, [[1, W]]),
                                    in_=zt[0:cr, 0:W])
                nc.scalar.dma_start(
                    out=sub_ap(bap, 0, cr, GUARD + (W - 1) * W, [[1, W]]),
                    in_=zt[0:cr, 0:W])

            # ---------------------------------------------------------------
            def punch(act_ap, p0, pstride, base_px, Q, W):
                """Zero halo-col positions inside a flat act range."""
                pitch = act_ap.ap[0][0]
                for edge in (0, W - 1):
                    o0 = (edge - base_px) % W
                    cnt = (Q - o0 + W - 1) // W if o0 < Q else 0
                    if cnt <= 0:
                        continue
                    ap = bass.AP(act_ap.tensor,
                                 act_ap.offset + p0 * pitch + o0,
                                 [[pitch, pstride], [W, cnt]])
                    nc.vector.memset(ap, 0.0)

            # --- s1: 3x3 stride-1 conv ------------------------------------
            def emit_s1(name, cin, cout, w, inb, outb):
                W = wp(w)
                blocks, mms = s1_plan(cin, w)
                nb = len(blocks)
                stride = pack_stride(cout)
                pack = 128 // stride
                total = w * W
                o_base = GUARD + W
                nchunks = (total + 511) // 512
                inap = xp_in if inb == "xp" else bufs[inb]
                in_pitch = inap.ap[0][0]
                bias = bslice(name)
                g_ch = 0
                while g_ch < nchunks:
                    nch = min(32, nchunks - g_ch)
                    nbk = min(pack, nch)
                    nchb = (nch + nbk - 1) // nbk
                    Q = nchb * 512
                    g = g_ch * 512
                    span = nbk * Q + 2 * W + 2
                    A0 = o_base + g - W - 1
                    ext = 0 if cin == 1 else blocks[-1]
                    tin = tinpool.tile([nb * cin, span + ext], STREAM_DT,
                                       tag="tin")
                    tin_ap = tin[:, :]
                    if cin == 1:
                        # xp9 rows are pre-shifted: one dense-partition load
                        nc.sync.dma_start(
                            out=tin[:, 0:span],
                            in_=dram_ap(inap, A0, [[in_pitch, 9], [1, span]]))
                    else:
                        # load block 0 (extended), replicate shifts on-chip
                        nc.sync.dma_start(
                            out=tin[0:cin, :],
                            in_=dram_ap(inap, A0,
                                        [[in_pitch, cin], [1, span + ext]]))
                        for b, s in enumerate(blocks[1:], start=1):
                            nc.sync.dma_start(
                                out=tin[b * cin:(b + 1) * cin, 0:span],
                                in_=tin[0:cin, s:s + span])

                    # act layout: block pk at partitions [pk*stride], free [0,Q)
                    act = actpool.tile([128, Q], STREAM_DT, tag="act")
                    act_ap = act[:, :]
                    npt = (Q + 1023) // 1024
                    for jt in range(npt):
                        ps = pspool.tile([128, 1024], F32, tag="ps")
                        nu = min(2, (Q - jt * 1024) // 512)
                        for u in range(nu):
                            for mi, m in enumerate(mms):
                                K = len(m["segs"]) * cin
                                for pk in range(nbk):
                                    foff = pk * Q + jt * 1024 + u * 512 + m["r"]
                                    rhs = sub_ap(tin_ap, 0, K, foff, [[1, 512]])
                                    nc.tensor.matmul(
                                        ps[pk * stride:pk * stride + cout,
                                           u * 512:u * 512 + 512],
                                        lhsT=wslice(f"W_{name}_{mi}"),
                                        rhs=rhs,
                                        start=(mi == 0), stop=(mi == len(mms) - 1),
                                        tile_position=(0, pk * stride))
                        tl = nu * 512
                        nc.scalar.activation(
                            sub_ap(act_ap, 0, 128, jt * 1024, [[1, tl]]),
                            ps[:, 0:tl], AF.Prelu, bias=bias, alpha=ALPHA)
                    for pk in range(nbk):
                        punch(act_ap, pk * stride, stride, g + pk * Q, Q, W)
                    # per-block stores: outer dim = cout spreads engines
                    dp = bufs[outb].ap[0][0]
                    for pk in range(nbk):
                        nc.gpsimd.dma_start(
                            out=dram_ap(bufs[outb], o_base + g + pk * Q,
                                        [[dp, cout], [1, Q]]),
                            in_=act[pk * stride:pk * stride + cout, 0:Q])
                    g_ch += nbk * nchb
                grid_end = g_ch * 512
                if grid_end > total:
                    ov = grid_end - total
                    ln = max(W, ov)
                    nc.scalar.dma_start(
                        out=sub_ap(bufs[outb], 0, cout, o_base + total, [[1, ln]]),
                        in_=zt[0:cout, 0:ln])

            # --- s2: 3x3 stride-2 conv ------------------------------------
            def emit_s2(name, cin, cout, w_in, w_out, inb, outb):
                Wi, Wo = wp(w_in), wp(w_out)
                blocks, mms = s2_plan(cin, w_in)
                nb = len(blocks)
                stride = pack_stride(cout)
                pack = 128 // stride
                Rq = 1024 // w_out
                Rch = 512 // w_out
                SR = pack * Rq
                L = Rq * Wo
                in_pitch = bufs[inb].ap[0][0]
                bias = bslice(name)
                for y0 in range(0, w_out, SR):
                    A0 = GUARD + 2 * y0 * Wi
                    span = (2 * SR + 2) * Wi
                    ext = blocks[-1]
                    tin = tinpool.tile([nb * cin, span + ext], STREAM_DT,
                                       tag="tin")
                    tin_ap = tin[:, :]
                    nc.sync.dma_start(
                        out=tin[0:cin, :],
                        in_=dram_ap(bufs[inb], A0,
                                    [[in_pitch, cin], [1, span + ext]]))
                    for b, s in enumerate(blocks[1:], start=1):
                        nc.sync.dma_start(
                            out=tin[b * cin:(b + 1) * cin, 0:span],
                            in_=tin[0:cin, s:s + span])

                    act = actpool.tile([128, L], STREAM_DT, tag="act")
                    act_ap = act[:, :]
                    ps = pspool.tile([128, 1024], F32, tag="ps")
                    for u in range(2):
                        for mi, m in enumerate(mms):
                            K = len(m["segs"]) * cin
                            for pk in range(pack):
                                foff = 2 * (pk * Rq + u * Rch) * Wi + m["r"]
                                rhs = sub_ap(tin_ap, 0, K, foff,
                                             [[2 * Wi, Rch], [2, w_out]])
                                nc.tensor.matmul(
                                    ps[pk * stride:pk * stride + cout,
                                       u * 512:u * 512 + 512],
                                    lhsT=wslice(f"W_{name}_{mi}"),
                                    rhs=rhs,
                                    start=(mi == 0), stop=(mi == len(mms) - 1),
                                    tile_position=(0, pk * stride))
                    nc.scalar.activation(
                        bass.AP(act_ap.tensor, act_ap.offset + 1,
                                [[act_ap.ap[0][0], 128], [Wo, Rq], [1, w_out]]),
                        ps[:, 0:1024], AF.Prelu, bias=bias, alpha=ALPHA)
                    nc.vector.memset(
                        bass.AP(act_ap.tensor, act_ap.offset,
                                [[act_ap.ap[0][0], 128], [Wo, Rq]]), 0.0)
                    nc.vector.memset(
                        bass.AP(act_ap.tensor, act_ap.offset + Wo - 1,
                                [[act_ap.ap[0][0], 128], [Wo, Rq]]), 0.0)
                    dp = bufs[outb].ap[0][0]
                    for pk in range(pack):
                        nc.gpsimd.dma_start(
                            out=dram_ap(bufs[outb],
                                        GUARD + (1 + y0 + pk * Rq) * Wo,
                                        [[dp, cout], [1, L]]),
                            in_=act[pk * stride:pk * stride + cout, 0:L])

            # --- tconv: stride-2 transposed conv + skip-add ---------------
            def emit_tconv(name, cin, cout, w_in, w_out, inb, outb, skipb):
                Wi, Wop = wp(w_in), wp(w_out)
                blocks, classes = tconv_plan(cin, w_in)
                nb = len(blocks)
                stride = pack_stride(cout)
                pack = 128 // stride
                Ri = 512 // w_in
                Rq = 1024 // w_in
                SR = pack * Rq
                L = 2 * Rq * Wop
                in_pitch = bufs[inb].ap[0][0]
                sk_pitch = bufs[skipb].ap[0][0]
                bias = bslice(name)
                for i0 in range(0, w_in, SR):
                    A0 = GUARD + (1 + i0) * Wi + 1
                    span = (SR + 2) * Wi
                    ext = blocks[-1]
                    tin = tinpool.tile([nb * cin, span + ext], STREAM_DT,
                                       tag="tin")
                    tin_ap = tin[:, :]
                    nc.sync.dma_start(
                        out=tin[0:cin, :],
                        in_=dram_ap(bufs[inb], A0,
                                    [[in_pitch, cin], [1, span + ext]]))
                    for b, s in enumerate(blocks[1:], start=1):
                        nc.sync.dma_start(
                            out=tin[b * cin:(b + 1) * cin, 0:span],
                            in_=tin[0:cin, s:s + span])

                    slab = actpool.tile([128, L], STREAM_DT, tag="act")
                    slab_ap = slab[:, :]
                    spitch = slab_ap.ap[0][0]
                    for py in range(2):
                        for px in range(2):
                            mms = [m for (py_, px_, cmms) in classes
                                   if (py_, px_) == (py, px) for m in cmms]
                            ps = pspool.tile([128, 1024], F32, tag="ps")
                            for u in range(2):
                                for mi, m in enumerate(mms):
                                    K = len(m["segs"]) * cin
                                    for pk in range(pack):
                                        foff = (pk * Rq + u * Ri) * Wi + m["r"]
                                        rhs = sub_ap(tin_ap, 0, K, foff,
                                                     [[Wi, Ri], [1, w_in]])
                                        nc.tensor.matmul(
                                            ps[pk * stride:pk * stride + cout,
                                               u * 512:u * 512 + 512],
                                            lhsT=wslice(f"W_{name}_c{py}{px}_{mi}"),
                                            rhs=rhs,
                                            start=(mi == 0),
                                            stop=(mi == len(mms) - 1),
                                            tile_position=(0, pk * stride))
                            nc.scalar.activation(
                                bass.AP(slab_ap.tensor,
                                        slab_ap.offset + py * Wop + 1 + px,
                                        [[spitch, 128], [2 * Wop, Rq], [2, w_in]]),
                                ps[:, 0:1024], AF.Prelu, bias=bias, alpha=ALPHA)
                    nc.vector.memset(
                        bass.AP(slab_ap.tensor, slab_ap.offset,
                                [[spitch, 128], [Wop, 2 * Rq]]), 0.0)
                    nc.vector.memset(
                        bass.AP(slab_ap.tensor, slab_ap.offset + Wop - 1,
                                [[spitch, 128], [Wop, 2 * Rq]]), 0.0)
                    skt = skpool.tile([128, L], STREAM_DT, tag="skt")
                    for pk in range(pack):
                        off = GUARD + (1 + 2 * (i0 + pk * Rq)) * Wop
                        nc.scalar.dma_start(
                            out=skt[pk * stride:pk * stride + cout, 0:L],
                            in_=dram_ap(bufs[skipb], off,
                                        [[sk_pitch, cout], [1, L]]))
                    nc.vector.tensor_add(out=slab_ap, in0=slab_ap, in1=skt[:, :])
                    dp = bufs[outb].ap[0][0]
                    for pk in range(pack):
                        off = GUARD + (1 + 2 * (i0 + pk * Rq)) * Wop
                        nc.gpsimd.dma_start(
                            out=dram_ap(bufs[outb], off, [[dp, cout], [1, L]]),
                            in_=slab[pk * stride:pk * stride + cout, 0:L])

            for (name, kind, cin, cout, w_in, w_out, wsrc, inb, outb, skipb) \
                    in LAYERS[:nlayers]:
                if kind == "s1":
                    emit_s1(name, cin, cout, w_in, inb, outb)
                elif kind == "s2":
                    emit_s2(name, cin, cout, w_in, w_out, inb, outb)
                else:
                    emit_tconv(name, cin, cout, w_in, w_out, inb, outb, skipb)

            # --- final 1x1 conv + triangular masking ----------------------
            do_tail = nlayers > len(LAYERS)
            if not do_tail:
                nc.sync.dma_start(out=out_t[:, :], in_=mask_t[:, :])
            W0 = wp(512)
            if do_tail:
                bias = bslice("out")
                dp = bufs["dec0p"].ap[0][0]
                for y0 in range(0, 512, 32):
                    A0 = GUARD + (1 + y0) * W0 + 1
                    span = 31 * W0 + 512
                    tin = tinpool.tile([16, span], STREAM_DT, tag="tin")
                    tin_ap = tin[:, :]
                    nc.sync.dma_start(out=tin_ap,
                                      in_=dram_ap(bufs["dec0p"], A0,
                                                  [[dp, 16], [1, span]]))
                    for t in range(4):
                        ps = pspool.tile([128, 1024], F32, tag="ps")
                        for u in range(2):
                            for pk in range(4):
                                r = t * 8 + pk * 2 + u
                                rhs = sub_ap(tin_ap, 0, 16, r * W0, [[1, 512]])
                                nc.tensor.matmul(
                                    ps[pk * 32:pk * 32 + 1,
                                       u * 512:u * 512 + 512],
                                    lhsT=wslice("W_out"), rhs=rhs,
                                    start=True, stop=True,
                                    tile_position=(0, pk * 32))
                        act = actpool.tile([128, 1024], F32, tag="actf")
                        nc.scalar.activation(act[:, :], ps[:, :], AF.Identity,
                                             bias=bias)
                        mt = skpool.tile([128, 1024], F32, tag="mask")
                        mt_ap = mt[:, :]
                        mpitch = mt_ap.ap[0][0]
                        nc.scalar.dma_start(
                            out=bass.AP(mt_ap.tensor, mt_ap.offset,
                                        [[32 * mpitch, 4], [512, 2], [1, 512]]),
                            in_=dram_ap(mask_t, (y0 + t * 8) * 512,
                                        [[1024, 4], [512, 2], [1, 512]]))
                        nc.vector.tensor_mul(out=act[:, :], in0=act[:, :],
                                             in1=mt_ap)
                        a_ap = act[:, :]
                        apitch = a_ap.ap[0][0]
                        nc.gpsimd.dma_start(
                            out=dram_ap(out_t, (y0 + t * 8) * 512,
                                        [[1024, 4], [512, 2], [1, 512]]),
                            in_=bass.AP(a_ap.tensor, a_ap.offset,
                                        [[32 * apitch, 4], [512, 2], [1, 512]]))

            # --- diagonal softplus patch ----------------------------------
            do_diag = nlayers > len(LAYERS) + 1
            if do_diag:
                out_flat = out_t.flatten()
                diag_ap = bass.AP(out_flat.tensor, out_flat.offset, [[513, 512]])
                dt_ = actpool.tile([1, 512], F32, tag="diag")
                nc.sync.dma_start(out=dt_[:, :], in_=diag_ap)
                ta = actpool.tile([1, 512], F32, tag="diag_a")
                nc.scalar.activation(ta[:, :], dt_[:, :], AF.Abs)
                nc.scalar.activation(ta[:, :], ta[:, :], AF.Exp, scale=-1.0)
                nc.vector.tensor_scalar_add(out=ta[:, :], in0=ta[:, :], scalar1=1.0)
                nc.scalar.activation(ta[:, :], ta[:, :], AF.Ln)
                tr = actpool.tile([1, 512], F32, tag="diag_r")
                nc.scalar.activation(tr[:, :], dt_[:, :], AF.Relu)
                nc.vector.tensor_add(out=tr[:, :], in0=tr[:, :], in1=ta[:, :])
                nc.sync.dma_start(out=diag_ap, in_=tr[:, :])

    nc.compile()
    return nc


_NC_CACHE = None


def get_nc():
    global _NC_CACHE
    if _NC_CACHE is None:
        _NC_CACHE = build_unet()
    return _NC_CACHE


def make_in_maps(inputs):
    wmap = prep_weights(inputs)
    x = np.asarray(inputs["x"])  # [8, 512, 512, 1] f32
    in_maps = []
    for i in range(B):
        m = dict(wmap)
        m["xp"] = prep_x(x[i, :, :, 0])
        in_maps.append(m)
    return in_maps


def kernel(_trace=False, **inputs):
    nc = get_nc()
    in_maps = make_in_maps(inputs)
    res = run_bass_kernel_spmd(nc, in_maps, core_ids=list(range(B)),
                               trace=_trace)
    out = np.stack([res.results[i]["out"] for i in range(B)], axis=0)
    out = out[:, :, :, None].astype(np.float32)
    if _trace:
        return out, res
    return out
